# revision 2
# baseline (speedup 1.0000x reference)
"""Trainium2 Bass kernel for nn_Attention_17016660426876.

Full-input contract: kernel(**inputs) takes unsharded inputs, returns the
full (4, 2048, 1024) fp32 output. 8 cores: core c = (batch b=c//2,
head-half hh=c%2); each core computes 8 heads of one batch and a partial
output projection; host sums core pairs.

Measured 487.7us on HW (vs 577.3us prior), rel err 1.49e-2.

Key structure:
- bf16 inputs/QKV matmuls (1 cyc/row like fp32r, half the DMA bytes);
  output projection in fp32r for precision at identical PE cost
- phase 1 rope/rmsnorm spread across engines to stay under the HAM
  power cap: Act evacuates PSUM + sqrt, DVE does 4x-bf16 muls + recip,
  Pool takes square/add; rotation via a +-1 permutation matmul
- phase 2: scores -> exp -> PV per kc tile; exp mostly on the Act
  engine (its 1337ns/tile stream is the phase bound), with 2 of every
  16 kc tiles computed on DVE via a Schraudolph bits-trick straight to
  bf16 (error centered by HW round-to-nearest, softmax cancels bias)
- PV stationary is [v(64) | ones(64)] so the softmax denominator comes
  out as 64 duplicated PSUM rows in the same matmul: full-width
  reciprocal+multiply on DVE, no partition broadcast
- PSUM: 2x double-buffered score tiles + 2x num/den accumulators fill
  all 8 banks; PV lags scores by 3 kc so the division pipeline hides
"""

import sys

sys.path.insert(0, "/opt/trn_rl_repo")

from contextlib import ExitStack

import numpy as np
import ml_dtypes

import concourse.bass as bass
import concourse.mybir as mybir
import concourse.tile as tile
from concourse import bacc
from concourse.bass_utils import run_bass_kernel_spmd

import os
DEBUG_DUMP = bool(os.environ.get("KDEBUG"))

B, N, C, H, D = 4, 2048, 1024, 16, 64
NCORES = 8
HL = H // 2
CL = HL * D
F_QK = 2 * CL

F32 = mybir.dt.float32
F32R = mybir.dt.float32r
BF16 = mybir.dt.bfloat16
I16 = mybir.dt.int16

ALU = mybir.AluOpType
ACT = mybir.ActivationFunctionType

# ---- exp config (bf16 P) ----
# P = exp(0.125*s) in bf16. Most kc tiles use the Act engine's exact
# exp; a minority go to DVE via the schraudolph bits trick straight to
# bf16 (int16 bits, always in-range since bf16 exp bias 127 >> |l|).
# The DVE sawtooth (~+-4%) only touches its minority share of softmax
# mass; v and P are bf16 so no fp8 content error.
LOG2E = 1.4426950408889634
SC0 = 0.125 * LOG2E * 128.0      # schraudolph scale on raw scores
SC1 = 127.0 * 128.0              # bf16 exponent bias << 7
SIG = 0.0

# per-16-kc engine pattern: 1 = Act exp, 0 = DVE schraudolph (13:3)
import os as _os
ACT_PAT = [1]*16 if _os.environ.get('ALLACT') else [1, 1, 1, 1, 1, 1, 1, 0, 1, 1, 1, 1, 1, 1, 1, 0]
ACT_PAT2 = ACT_PAT

PVLAG = 3                        # kc lag of PV behind S/exp (hides div)


def build_nc():
    nc = bacc.Bacc("TRN2", target_bir_lowering=False, debug=False,
                   num_devices=NCORES)

    xT = nc.dram_tensor("xT", [C, N], BF16, kind="ExternalInput")
    wq = nc.dram_tensor("wq", [8, 128, F_QK], BF16, kind="ExternalInput")
    wvT = nc.dram_tensor("wvT", [C, CL], BF16, kind="ExternalInput")
    wpT = nc.dram_tensor("wpT", [CL, C], F32R, kind="ExternalInput")
    tab_cq = nc.dram_tensor("tab_cq", [128, N], BF16, kind="ExternalInput")
    tab_sq = nc.dram_tensor("tab_sq", [128, N], BF16, kind="ExternalInput")
    tab_ck = nc.dram_tensor("tab_ck", [128, N], BF16, kind="ExternalInput")
    tab_sk = nc.dram_tensor("tab_sk", [128, N], BF16, kind="ExternalInput")
    blk = nc.dram_tensor("blk", [128, 128], BF16, kind="ExternalInput")
    rmat = nc.dram_tensor("rmat", [128, 128], BF16, kind="ExternalInput")
    out = nc.dram_tensor("out", [N, C], F32, kind="ExternalOutput")
    if DEBUG_DUMP:
        dbg_qkT0 = nc.dram_tensor("dbg_qkT0", [128, N], F32, kind="ExternalOutput")
        dbg_qkT4 = nc.dram_tensor("dbg_qkT4", [128, N], F32, kind="ExternalOutput")
        dbg_v5 = nc.dram_tensor("dbg_v5", [128, 16 * 512], F32, kind="ExternalOutput")
        dbg_aT0 = nc.dram_tensor("dbg_aT0", [128, N], F32, kind="ExternalOutput")
        dbg_nd = nc.dram_tensor("dbg_nd", [128, 1024], F32, kind="ExternalOutput")
        dbg_pt = nc.dram_tensor("dbg_pt", [128, 1024], F32, kind="ExternalOutput")
        dbg_p1 = {n: nc.dram_tensor(f"dbg_{n}", [128, 512], F32,
                                    kind="ExternalOutput")
                  for n in ("raw", "sq", "rb", "u", "w", "v2")}

    with tile.TileContext(nc) as tc, ExitStack() as top:
        pers = top.enter_context(tc.tile_pool(name="pers", bufs=1))

        # long-lived SBUF
        qkT = [pers.tile([128, N], BF16, name=f"qkT{j}") for j in range(8)]
        # v5: per (kc, hd): 64 v cols then 64 ones cols -- the dup-row
        # denominator rides each PV matmul stationary for free
        v5 = pers.tile([128, 16 * 8 * 128], BF16, name="v5")
        wvT_sb = [pers.tile([128, CL], BF16, name=f"wvT{i}") for i in range(8)]
        blk_sb = pers.tile([128, 128], BF16, name="blk_sb")
        rmat_sb = pers.tile([128, 128], BF16, name="rmat_sb")
        tabs_sb = {}
        for nm, dr_ in (("cq", tab_cq), ("sq", tab_sq),
                        ("ck", tab_ck), ("sk", tab_sk)):
            t = pers.tile([128, N], BF16, name=f"tab_{nm}")
            tabs_sb[nm] = t
        c1t = pers.tile([128, 1024], F32, name="c1t")
        warm = pers.tile([128, 128], BF16, name="warm")

        nc.vector.memset(c1t, SC1 + SIG)
        nc.vector.memset(warm, 0.001)
        nc.vector.memset(v5, 1.0)

        # PE warm-up while first DMAs land (no DMA dependency)
        with tc.tile_pool(name="warmp", bufs=1, space="PSUM") as warmp:
            wps = warmp.tile([128, 128], F32, tag="warm", name="warm_ps")
            for i in range(40):
                nc.tensor.matmul(wps, warm, warm, start=True, stop=True)

        for nm, dr_ in (("cq", tab_cq), ("sq", tab_sq),
                        ("ck", tab_ck), ("sk", tab_sk)):
            nc.sync.dma_start(out=tabs_sb[nm], in_=dr_[:, :])
        for i in range(8):
            nc.sync.dma_start(out=wvT_sb[i], in_=wvT[i * 128:(i + 1) * 128, :])
        nc.sync.dma_start(out=blk_sb, in_=blk[:, :])
        nc.sync.dma_start(out=rmat_sb, in_=rmat[:, :])

        # ---------------- phase 1: qkv + rmsnorm + rope ----------------
        with ExitStack() as p1:
            xp = p1.enter_context(tc.tile_pool(name="xp", bufs=2))
            wqp = p1.enter_context(tc.tile_pool(name="wqp", bufs=3))
            scr = p1.enter_context(tc.tile_pool(name="scr", bufs=2))
            pqk = p1.enter_context(tc.tile_pool(name="pqk", bufs=4, space="PSUM"))
            pv = p1.enter_context(tc.tile_pool(name="pv", bufs=2, space="PSUM"))
            pm = p1.enter_context(tc.tile_pool(name="pm", bufs=1, space="PSUM"))
            prot = p1.enter_context(tc.tile_pool(name="prot", bufs=1, space="PSUM"))

            for th in range(2):
                xq, tsq = [], []
                for qq in range(2):
                    tq = th * 2 + qq
                    ts = slice(tq * 512, tq * 512 + 512)
                    tsq.append(ts)
                    xts = []
                    for ci in range(8):
                        t = xp.tile([128, 512], BF16, tag=f"x{ci}",
                                    name=f"x{ci}_{tq}")
                        nc.sync.dma_start(out=t, in_=xT[ci * 128:(ci + 1) * 128, ts])
                        xts.append(t)
                    xq.append(xts)

                    # v: (tokens, feature) tiles -> v5 fp8
                    for tk in range(4):
                        tg = tq * 4 + tk
                        ps = pv.tile([128, CL], F32, tag="pv", name=f"pv{tg}")
                        for ci in range(8):
                            nc.tensor.matmul(
                                ps, xts[ci][:, tk * 128:(tk + 1) * 128],
                                wvT_sb[ci], start=(ci == 0), stop=(ci == 7))
                        v5t = v5[:, tg * 1024:(tg + 1) * 1024].rearrange(
                            "p (h two d) -> p h two d", h=HL, two=2)
                        nc.scalar.copy(
                            out=v5t[:, :, 0, :],
                            in_=ps.rearrange("p (h d) -> p h d", h=HL))

                # q,k per j: two token-quarters share the weight loads
                for j in (0, 4, 1, 5, 2, 6, 3, 7):
                    wqt = wqp.tile([128, F_QK], BF16, tag="wq", name=f"wq{j}_{th}")
                    nc.sync.dma_start(out=wqt, in_=wq[j])
                    ps2 = [pqk.tile([128, 512], F32, tag="pqk",
                                    name=f"pqk{j}_{th}_{qq}") for qq in range(2)]
                    for ci in range(8):
                        for qq in range(2):
                            nc.tensor.matmul(
                                ps2[qq], wqt[:, ci * 128:(ci + 1) * 128],
                                xq[qq][ci], start=(ci == 0), stop=(ci == 7))
                    tc_, tss = (tabs_sb["cq"], tabs_sb["sq"]) if j < 4 else \
                               (tabs_sb["ck"], tabs_sb["sk"])
                    for qq in range(2):
                        ts = tsq[qq]
                        raw = scr.tile([128, 512], BF16, tag="raw",
                                       name=f"raw{j}_{th}_{qq}")
                        nc.scalar.copy(out=raw, in_=ps2[qq])
                        sq = scr.tile([128, 512], BF16, tag="sq",
                                      name=f"sq{j}_{th}_{qq}")
                        nc.gpsimd.tensor_mul(sq, raw, raw)
                        psm = pm.tile([128, 512], F32, tag="pm",
                                      name=f"pm{j}_{th}_{qq}")
                        nc.tensor.matmul(psm, blk_sb, sq, start=True, stop=True)
                        srt = scr.tile([128, 512], F32, tag="srt",
                                       name=f"srt{j}_{th}_{qq}")
                        nc.scalar.activation(srt, psm, ACT.Sqrt, scale=1.0)
                        rmi = scr.tile([128, 512], F32, tag="rmi",
                                       name=f"rmi{j}_{th}_{qq}")
                        nc.vector.reciprocal_approx_fast(out=rmi, in_=srt)
                        rb = scr.tile([128, 512], BF16, tag="rb",
                                      name=f"rb{j}_{th}_{qq}")
                        nc.scalar.copy(out=rb, in_=rmi)
                        u = scr.tile([128, 512], BF16, tag="u",
                                     name=f"u{j}_{th}_{qq}")
                        nc.vector.tensor_mul(u, raw, tc_[:, ts])
                        prs = prot.tile([128, 512], F32, tag="prot",
                                        name=f"prot{j}_{th}_{qq}")
                        nc.tensor.matmul(prs, rmat_sb, raw, start=True,
                                         stop=True)
                        w = scr.tile([128, 512], BF16, tag="w",
                                     name=f"w{j}_{th}_{qq}")
                        nc.vector.tensor_mul(w, prs, tss[:, ts])
                        v2 = scr.tile([128, 512], BF16, tag="v2",
                                      name=f"v2{j}_{th}_{qq}")
                        nc.gpsimd.tensor_add(v2, u, w)
                        nc.vector.tensor_mul(qkT[j][:, ts], v2, rb)

        # ---------------- phase 2: attention, head pairs ----------------
        with ExitStack() as p2:
            wpp = p2.enter_context(tc.tile_pool(name="wpp", bufs=1))
            wpT_sb = [wpp.tile([128, C], F32R, name=f"wpT{i}") for i in range(4)]
            for i in range(4):
                nc.sync.dma_start(out=wpT_sb[i], in_=wpT[i * 128:(i + 1) * 128, :])
            aT = [wpp.tile([128, N], F32R, name=f"aT{i}") for i in range(4)]

            with ExitStack() as p2i:
                ptp = p2i.enter_context(tc.tile_pool(name="ptp", bufs=5))
                dvs = p2i.enter_context(tc.tile_pool(name="dvs", bufs=2))
                sps = p2i.enter_context(tc.tile_pool(name="sps", bufs=2, space="PSUM"))
                ndp = p2i.enter_context(tc.tile_pool(name="ndp", bufs=2, space="PSUM"))

                for hd in range(8):
                    j, row = hd // 2, (hd % 2) * 64
                    qt = qkT[j][row:row + 64, :]
                    kt = qkT[j + 4][row:row + 64, :]
                    pat = ACT_PAT if hd % 2 == 0 else ACT_PAT2
                    for qh in range(2):
                        qbase = qh * 1024
                        nd = ndp.tile([128, 1024], F32, tag="nd",
                                      name=f"nd{hd}_{qh}")
                        pts = []
                        for kc in range(16):
                            sp = sps.tile([128, 1024], F32, tag="sp",
                                          name=f"sp{hd}_{qh}_{kc}")
                            for q2 in range(2):
                                qs = slice(qbase + q2 * 512,
                                           qbase + q2 * 512 + 512)
                                nc.tensor.matmul(
                                    sp[:, q2 * 512:q2 * 512 + 512],
                                    kt[:, kc * 128:(kc + 1) * 128],
                                    qt[:, qs], start=True, stop=True)
                            pt = ptp.tile([128, 1024], BF16, tag="pt",
                                          name=f"pt{hd}_{qh}_{kc}")
                            pts.append(pt)
                            if pat[kc]:
                                nc.scalar.activation(pt, sp, ACT.Exp,
                                                     scale=0.125)
                            else:
                                nc.vector.scalar_tensor_tensor(
                                    out=pt.bitcast(I16), in0=sp,
                                    scalar=SC0, in1=c1t,
                                    op0=ALU.mult, op1=ALU.add)
                            if kc >= PVLAG:
                                emit_pv(nc, pts[kc - PVLAG], v5, nd, hd,
                                        kc - PVLAG)
                        for kc in range(16 - PVLAG, 16):
                            emit_pv(nc, pts[kc], v5, nd, hd, kc)
                        # softmax division
                        dsb = dvs.tile([64, 1024], F32, tag="dsb",
                                       name=f"dsb{hd}_{qh}")
                        nc.vector.tensor_copy(out=dsb, in_=nd[64:128, :])
                        rden = dvs.tile([64, 1024], F32, tag="rden",
                                        name=f"rden{hd}_{qh}")
                        nc.vector.reciprocal_approx_fast(out=rden, in_=dsb)
                        nc.vector.tensor_mul(
                            aT[hd // 2][row:row + 64, qbase:qbase + 1024],
                            nd[0:64, :], rden)

            # ---------------- phase 3: output projection ----------------
            with ExitStack() as p3:
                osb = p3.enter_context(tc.tile_pool(name="osb", bufs=3))
                ppj = p3.enter_context(tc.tile_pool(name="ppj", bufs=2, space="PSUM"))
                for tk in range(16):
                    pp = ppj.tile([128, C], F32, tag="pp", name=f"pp{tk}")
                    for ci in range(4):
                        for oh in range(2):
                            nc.tensor.matmul(
                                pp[:, oh * 512:oh * 512 + 512],
                                aT[ci][:, tk * 128:(tk + 1) * 128],
                                wpT_sb[ci][:, oh * 512:oh * 512 + 512],
                                start=(ci == 0), stop=(ci == 3),
                                skip_group_check=True)
                    ot = osb.tile([128, C], F32, tag="ot", name=f"ot{tk}")
                    nc.vector.tensor_copy(out=ot, in_=pp)
                    nc.sync.dma_start(out=out[tk * 128:(tk + 1) * 128, :], in_=ot)
                if DEBUG_DUMP:
                    for nm_, src_ in (("dbg_qkT0", qkT[0]), ("dbg_qkT4", qkT[4]),
                                      ("dbg_aT0", aT[0])):
                        d = osb.tile([128, N], F32, tag="dbg", name=f"d{nm_}")
                        nc.vector.tensor_copy(out=d, in_=src_)
                        nc.sync.dma_start(
                            out={"dbg_qkT0": dbg_qkT0, "dbg_qkT4": dbg_qkT4,
                                 "dbg_aT0": dbg_aT0}[nm_][:, :], in_=d)
                    for half in range(2):
                        d = osb.tile([128, 4096], F32, tag="dbgv",
                                     name=f"dv5_{half}")
                        nc.vector.tensor_copy(
                            out=d, in_=v5[:, half * 4096:(half + 1) * 4096])
                        nc.sync.dma_start(
                            out=dbg_v5[:, half * 4096:(half + 1) * 4096], in_=d)

    nc.compile()
    return nc


def emit_pv(nc, pt, v5, nd, hd, kc):
    """PV for one kc: stationary [128 tok, (v 64 | ones 64)] gives num in
    psum rows 0:64 and the duplicated denominator in rows 64:128."""
    vslice = v5[:, (kc * 8 + hd) * 128:(kc * 8 + hd) * 128 + 128]
    for q2 in range(2):
        nc.tensor.matmul(
            nd[:, q2 * 512:(q2 + 1) * 512],
            vslice, pt[:, q2 * 512:(q2 + 1) * 512],
            start=(kc == 0), stop=(kc == 15), skip_group_check=True)


def prep_inputs(x, cos, sin, w_qkv, w_proj, q_gamma, k_gamma):
    bf = ml_dtypes.bfloat16
    x = np.asarray(x, np.float32)
    cos = np.asarray(cos, np.float32)
    sin = np.asarray(sin, np.float32)
    w_qkv = np.asarray(w_qkv, np.float32)
    w_proj = np.asarray(w_proj, np.float32)
    q_gamma = np.asarray(q_gamma, np.float32)
    k_gamma = np.asarray(k_gamma, np.float32)

    cosT = np.ascontiguousarray(cos[0, 0].T)      # (64, N)
    sinT = np.ascontiguousarray(sin[0, 0].T)

    def tables(g):
        g_swap = g.reshape(D // 2, 2)[:, ::-1].reshape(D)
        ct = np.tile(cosT * g[:, None], (2, 1))
        st = np.tile(sinT * g_swap[:, None], (2, 1))
        return (np.ascontiguousarray(ct).astype(bf),
                np.ascontiguousarray(st).astype(bf))

    cq_t, sq_t = tables(q_gamma)
    ck_t, sk_t = tables(k_gamma)

    blk = np.zeros((128, 128), np.float32)
    blk[:64, :64] = 1.0 / 64
    blk[64:, 64:] = 1.0 / 64
    blk = blk.astype(bf)

    rmat = np.zeros((128, 128), np.float32)
    idx = np.arange(0, 128, 2)
    rmat[idx, idx + 1] = 1.0
    rmat[idx + 1, idx] = -1.0
    rmat = rmat.astype(bf)

    in_maps = []
    for c in range(NCORES):
        b, hh = c // 2, c % 2
        xT = np.ascontiguousarray(x[b].T).astype(bf)
        wq_rows = w_qkv[512 * hh:512 * hh + 512]
        wk_rows = w_qkv[1024 + 512 * hh:1024 + 512 * hh + 512]
        wv_rows = w_qkv[2048 + 512 * hh:2048 + 512 * hh + 512]
        wqkT = np.concatenate([wq_rows, wk_rows], 0).T   # (1024 c, 1024 f)
        wq_tiled = np.ascontiguousarray(
            wqkT.reshape(8, 128, 8, 128).transpose(2, 1, 0, 3)
            .reshape(8, 128, F_QK)).astype(bf)
        wvT = np.ascontiguousarray(wv_rows.T).astype(bf)
        wpT = np.ascontiguousarray(
            w_proj[:, 512 * hh:512 * hh + 512].T)
        in_maps.append({
            "xT": xT, "wq": wq_tiled, "wvT": wvT, "wpT": wpT,
            "tab_cq": cq_t, "tab_sq": sq_t, "tab_ck": ck_t, "tab_sk": sk_t,
            "blk": blk, "rmat": rmat,
        })
    return in_maps


_NC_CACHE = None


def get_nc():
    global _NC_CACHE
    if _NC_CACHE is None:
        _NC_CACHE = build_nc()
    return _NC_CACHE


def kernel(x, cos, sin, w_qkv, w_proj, q_gamma, k_gamma):
    nc = get_nc()
    in_maps = prep_inputs(x, cos, sin, w_qkv, w_proj, q_gamma, k_gamma)
    res = run_bass_kernel_spmd(nc, in_maps, list(range(NCORES)))
    parts = [res.results[c]["out"] for c in range(NCORES)]
    out = np.stack([parts[2 * b] + parts[2 * b + 1] for b in range(B)])
    return out.astype(np.float32)
